# revision 40
# baseline (speedup 1.0000x reference)
"""YOLO loss kernel for Trainium2 (Bass/Tile), data-parallel over 8 NeuronCores.

Math (per sample n, cell s; S=14, SS=196, B=2, C=20, D=30):
  t4 = target conf channel (binary 0/1, channels 4 and 9 identical)
  All box coords scaled by S (iou is invariant): corner = x +- (S/2)w,
  areas = S^2 * w * h.
  For the box PAIR (channels {o..o+3} for o in {0,5}, strided APs):
    prb = x + (S/2)w ; nlt = (S/2)w - x   (= -lt)
    ox = min(tprb, prb) + min(tnlt, nlt) ; oy likewise
    inter = relu(ox)*max(oy,0) ; union = S^2*pw*ph + S^2*tw*th - inter
    iou = inter * recip(union)
  sel = iou1 > iou0 ; selm = sel*t4 ; s0m = t4 - selm  (mask pair msk=[s0m,selm])
  conf  = sum msk_h*(p_{4+5h} - iou_h)^2          (pair op, channels {4,9})
  noobj = sum 0.5*(1-t4)*(p4^2 + p9^2)            (pair op)
  coord = 5 * sum_k msk_h*(p_{5h+k}-t_{5h+k})^2   (8-channel pair op, bcast msk)
  class = sum t4 * (p_c-t_c)^2, c=10..29          (4-channel chunks, bcast t4)
Masked squares use (mask*e)^2 = mask*e^2 (masks binary); weights fold into the
ACT Square scale. Every reduction is an ACT accum_out into a [128, NSLOT]
partial tile; host sums across slots/partitions/cores and divides by N.

Layout per core: 512 samples = 4 blocks x 128 partitions, processed in passes
of GRPS=[1,2,1] blocks (small first pass -> compute starts early; small last
pass + small trailing class chunks -> short drain tail). Engine split per the
TimelineSim cost model: DVE all stt/min/cmp + chain ops, Pool (gpsimd, 0.42
efficiency on add/sub/mult, nothing else supported) the independent big
sub/mults, ACT the square-accumulate reductions; the final chunk reduces via
mul+tensor_reduce on DVE to dodge the tail ACT queue. DMA stream is ordered so
it never stalls (65.8us = bytes/360GBps, gap-free): box data for passes 0/1,
class 0, one class-1 chunk, box data for pass 2, rest of class 1, class 2;
per-tag SBUF ring depths sized so no buffer-free wait ever blocks the queue.
"""

import numpy as np

import concourse.mybir as mybir
from concourse import bacc
from concourse.bass_utils import run_bass_kernel_spmd
from concourse.tile import TileContext

F32 = mybir.dt.float32
OP = mybir.AluOpType
AF = mybir.ActivationFunctionType

N, D, S = 4096, 30, 14
SS = S * S          # 196
NCORE = 8
NPC = N // NCORE    # 512 samples per core
P = 128
NBLK = NPC // P     # 4 blocks of 128 samples
GRPS = [1, 2, 1]    # blocks per pass
HALF = S / 2.0      # corner scale
AREA = float(S * S)
SQ5 = 5.0 ** 0.5
SQH = 0.5 ** 0.5

# class chunks (channel counts); last pass splits the tail chunk
CLS_CHUNKS = [[4, 4, 4, 4, 4], [4, 4, 4, 4, 4], [4, 4, 4, 4, 2, 2]]
BOX2_AFTER = 0
SMALL_BUFS = 3
PC1_BUFS = 5
TC1_BUFS = 5
TC2_BUFS = 4
COORD_SPLIT = ()
CLS0_EARLY = 0
BOX2_EMIT = 0
COORD_ENG = [("G", "V"), ("G", "V"), ("G", "V")]
NM_ENG = ["G", "G", "G"]
PQ_ENG = ["G", "G", "G"]
Q1_ENG = [("G", "V"), ("G", "V"), ("G", "G"), ("G", "V"), ("G", "V")]
Q2_ENG = [("G", "V"), ("G", "V"), ("G", "V"), ("G", "V"), ("G", "G"),
          ("V", "V", "V")]
SLOTS_PER_PASS = [3 + len(c) for c in CLS_CHUNKS]   # noobj, conf, coord + class
NSLOT = sum(SLOTS_PER_PASS)                          # 25

_CACHE = {}


def _build():
    nc = bacc.Bacc("TRN2", target_bir_lowering=False, debug=False)
    pred = nc.dram_tensor("pred", [NPC, D * SS], F32, kind="ExternalInput")
    tgt = nc.dram_tensor("target", [NPC, D * SS], F32, kind="ExternalInput")
    out = nc.dram_tensor("out", [P, NSLOT], F32, kind="ExternalOutput")

    # [NPC, D*SS] -> [P, block, D*SS]; sample = block*128 + p
    pred_r = pred[:, :].rearrange("(a p) d -> p a d", a=NBLK)
    tgt_r = tgt[:, :].rearrange("(a p) d -> p a d", a=NBLK)

    npass = len(GRPS)
    offs = [sum(GRPS[:q]) for q in range(npass)]

    with TileContext(nc) as tc:
        with (
            tc.tile_pool(name="box", bufs=1) as boxp,     # pb/tb per pass
            tc.tile_pool(name="cls", bufs=3) as clsp,     # class chunk streams
            tc.tile_pool(name="tmp", bufs=1) as tmp,      # per-pass temps
            tc.tile_pool(name="accp", bufs=1) as accp,
        ):
            acc = accp.tile([P, NSLOT], F32)

            pb, tb, t4t = [], [], []
            pc, tcl = [], []
            for q, g in enumerate(GRPS):
                pb.append(boxp.tile([P, g, 10, SS], F32, tag=f"pb{q}",
                                    name=f"pb{q}"))
                # target as 8-ch box tile {0..3,5..8} + separate t4 tile
                tb.append(boxp.tile([P, g, 2, 4, SS], F32, tag=f"tb{q}",
                                    name=f"tb{q}"))
                t4t.append(boxp.tile([P, g, SS], F32, tag=f"t4_{q}",
                                     name=f"t4_{q}"))
                pcq, tcq = [], []
                for j, w in enumerate(CLS_CHUNKS[q]):
                    pbufs = (PC1_BUFS if g == 1 else 3) if w == 4 else SMALL_BUFS
                    tbufs = (TC1_BUFS if g == 1 else TC2_BUFS) if w == 4 else SMALL_BUFS
                    pcq.append(clsp.tile([P, g, w, SS], F32, tag=f"pc{g}_{w}",
                                         name=f"pc{q}_{j}", bufs=pbufs))
                    tcq.append(clsp.tile([P, g, w, SS], F32, tag=f"tc{g}_{w}",
                                         name=f"tc{q}_{j}", bufs=tbufs))
                pc.append(pcq)
                tcl.append(tcq)

            def dma_box(q):
                g = GRPS[q]
                a0 = offs[q]
                tgt8 = tgt_r[:, a0:a0 + g, 0:10 * SS].rearrange(
                    "p g (h c s) -> p g h c s", h=2, c=5)
                for h in range(2):
                    nc.sync.dma_start(
                        out=tb[q][:, :, h, :, :], in_=tgt8[:, :, h, 0:4, :])
                nc.sync.dma_start(out=t4t[q],
                                  in_=tgt_r[:, a0:a0 + g, 4 * SS:5 * SS])
                nc.sync.dma_start(
                    out=pb[q], in_=pred_r[:, a0:a0 + g, 0:10 * SS])

            def dma_class(q, j, which="both"):
                g = GRPS[q]
                a0 = offs[q]
                lo = 10 + sum(CLS_CHUNKS[q][:j])
                w = CLS_CHUNKS[q][j]
                if which in ("both", "p"):
                    nc.sync.dma_start(
                        out=pc[q][j],
                        in_=pred_r[:, a0:a0 + g, lo * SS:(lo + w) * SS])
                if which in ("both", "t"):
                    nc.sync.dma_start(
                        out=tcl[q][j],
                        in_=tgt_r[:, a0:a0 + g, lo * SS:(lo + w) * SS])

            # DMA stream order: boxes q0/q1, class0, class1 (box q2
            # placed after class1 chunk BOX2_AFTER; -1 = upfront)
            dma_box(0)
            for j in range(CLS0_EARLY):
                dma_class(0, j)
            dma_box(1)
            if BOX2_AFTER < 0:
                dma_box(2)
            for j in range(CLS0_EARLY, len(CLS_CHUNKS[0])):
                dma_class(0, j)
            for j in range(len(CLS_CHUNKS[1])):
                dma_class(1, j)
                if j == BOX2_AFTER:
                    dma_box(2)
            for j in range(len(CLS_CHUNKS[2])):
                dma_class(2, j)

            # ---- per-pass state ----
            st = [dict() for _ in range(npass)]

            def slot(q, i):
                base = sum(SLOTS_PER_PASS[:q])
                return acc[:, base + i:base + i + 1]

            def T(q, name, shape):
                t = tmp.tile(shape, F32, tag=f"{name}g{GRPS[q]}",
                             name=f"{name}{q}")
                st[q][name] = t
                return t

            def box_phase(nc, q):
                g = GRPS[q]
                s = st[q]
                pbv = pb[q][:, :, :, :]
                tb0 = tb[q][:, :, 0, :, :]          # target box channels 0..3
                t4 = t4t[q][:, :, :]

                # target prep (DVE stt): scaled corners + area
                tpr = T(q, "tpr", [P, g, 2, SS])    # [x-axis, y-axis]
                tnl = T(q, "tnl", [P, g, 2, SS])
                nc.vector.scalar_tensor_tensor(
                    tpr, tb0[:, :, 2:4, :], HALF, tb0[:, :, 0:2, :],
                    OP.mult, OP.add)
                nc.vector.scalar_tensor_tensor(
                    tnl, tb0[:, :, 2:4, :], HALF, tb0[:, :, 0:2, :],
                    OP.mult, OP.subtract)
                tarea = T(q, "tarea", [P, g, SS])
                nc.vector.scalar_tensor_tensor(
                    tarea, tb0[:, :, 2, :], AREA, tb0[:, :, 3, :],
                    OP.mult, OP.mult)

                # w = 1 - t4 (ACT); noobj pair masked by w
                w = T(q, "w", [P, g, SS])
                nc.scalar.activation(w, t4, AF.Copy, bias=1.0, scale=-1.0)
                conf_p = pbv.rearrange("p g (h c) s -> p g h c s", h=2)[:, :, :, 4, :]
                nm = T(q, "ce", [P, g, 2, SS])
                wb2 = w[:, :, :].unsqueeze(2).broadcast_to([P, g, 2, SS])
                nm_eng = nc.gpsimd if NM_ENG[q] == "G" else nc.vector
                nm_eng.tensor_tensor(nm, conf_p, wb2, OP.mult)
                nc.scalar.activation(nm, nm, AF.Square, scale=SQH,
                                     accum_out=slot(q, 0))

                # pred corners, both boxes at once: channel pairs {c, c+5}
                pv = pbv.rearrange("p g (h c) s -> p g h c s", h=2)
                pw_x = pv[:, :, :, 2, :]   # [P, g, 2(box), SS]
                pw_y = pv[:, :, :, 3, :]
                px = pv[:, :, :, 0, :]
                py = pv[:, :, :, 1, :]
                prx = T(q, "prx", [P, g, 2, SS])
                nlx = T(q, "nlx", [P, g, 2, SS])
                pry = T(q, "pry", [P, g, 2, SS])
                nly = T(q, "nly", [P, g, 2, SS])
                nc.vector.scalar_tensor_tensor(prx, pw_x, HALF, px, OP.mult, OP.add)
                nc.vector.scalar_tensor_tensor(nlx, pw_x, HALF, px, OP.mult, OP.subtract)
                nc.vector.scalar_tensor_tensor(pry, pw_y, HALF, py, OP.mult, OP.add)
                nc.vector.scalar_tensor_tensor(nly, pw_y, HALF, py, OP.mult, OP.subtract)
                # pq = pred area (Pool): (S^2 folded in s1 stt via AREA scale)
                pq = T(q, "pq", [P, g, 2, SS])
                pq_eng = nc.gpsimd if PQ_ENG[q] == "G" else nc.vector
                pq_eng.tensor_tensor(pq, pw_x, pw_y, OP.mult)

                # intersect: mins (target side broadcast over box pair)
                tprxb = tpr[:, :, 0, :].unsqueeze(2).broadcast_to([P, g, 2, SS])
                tpryb = tpr[:, :, 1, :].unsqueeze(2).broadcast_to([P, g, 2, SS])
                tnlxb = tnl[:, :, 0, :].unsqueeze(2).broadcast_to([P, g, 2, SS])
                tnlyb = tnl[:, :, 1, :].unsqueeze(2).broadcast_to([P, g, 2, SS])
                nc.vector.tensor_tensor(prx, tprxb, prx, OP.min)
                nc.vector.tensor_tensor(nlx, tnlxb, nlx, OP.min)
                nc.vector.tensor_tensor(pry, tpryb, pry, OP.min)
                nc.vector.tensor_tensor(nly, tnlyb, nly, OP.min)
                nc.vector.tensor_add(prx, prx, nlx)          # ox
                nc.vector.tensor_add(pry, pry, nly)          # oy
                nc.scalar.activation(nlx, prx, AF.Relu)      # relu(ox)
                nc.vector.scalar_tensor_tensor(
                    pry, pry, 0.0, nlx, OP.max, OP.mult)     # inter
                # union = (S^2*pq - inter) + tarea
                tareab = tarea[:, :, :].unsqueeze(2).broadcast_to([P, g, 2, SS])
                nc.vector.scalar_tensor_tensor(
                    nly, pq, AREA, pry, OP.mult, OP.subtract)
                nc.vector.tensor_tensor(nly, nly, tareab, OP.add)    # union
                nc.vector.reciprocal_approx_fast(
                    out=prx[:, :, :, :].rearrange("p g h s -> p (g h s)"),
                    in_=nly[:, :, :, :].rearrange("p g h s -> p (g h s)"))
                iou = pq
                st[q]["iou"] = iou
                nc.vector.tensor_mul(iou, pry, prx)

                # masks
                sel = tarea
                nc.vector.tensor_tensor(
                    sel, iou[:, :, 1, :], iou[:, :, 0, :], OP.is_gt)
                msk = s["tpr"]
                st[q]["msk"] = msk
                nc.vector.tensor_mul(msk[:, :, 1, :], sel, t4)       # selm
                nc.vector.tensor_sub(msk[:, :, 0, :], t4, msk[:, :, 1, :])  # s0m

                # conf pair: (p_conf - iou) * msk
                ce = s["ce"]
                nc.vector.tensor_sub(ce, conf_p, iou)
                nc.vector.tensor_mul(ce, ce, msk)
                nc.scalar.activation(ce, ce, AF.Square, scale=1.0,
                                     accum_out=slot(q, 1))

            def coord_phase(nc, q, sub_eng, mul_eng):
                g = GRPS[q]
                s = st[q]
                pv8 = pb[q][:, :, :, :].rearrange(
                    "p g (h c) s -> p g h c s", h=2)[:, :, :, 0:4, :]
                tv8 = tb[q][:, :, :, :, :]
                e8 = T(q, "e8", [P, g, 2, 4, SS])
                mskb = s["msk"][:, :, :, :].unsqueeze(3).broadcast_to(
                    [P, g, 2, 4, SS])
                if q in COORD_SPLIT:
                    # independent half-chains (h=0: s0m/box0, h=1: selm/box1)
                    for h in range(2):
                        sub_eng.tensor_tensor(
                            e8[:, :, h, :, :], pv8[:, :, h, :, :],
                            tv8[:, :, h, :, :], OP.subtract)
                        mul_eng.tensor_tensor(
                            e8[:, :, h, :, :], e8[:, :, h, :, :],
                            mskb[:, :, h, :, :], OP.mult)
                        nc.scalar.activation(
                            e8[:, :, h, :, :], e8[:, :, h, :, :], AF.Square,
                            scale=SQ5, accum_out=slot(q, 2) if h == 0
                            else acc[:, NSLOT - 1 - q:NSLOT - q])
                else:
                    sub_eng.tensor_tensor(e8, pv8, tv8, OP.subtract)
                    mul_eng.tensor_tensor(e8, e8, mskb, OP.mult)
                    nc.scalar.activation(e8, e8, AF.Square, scale=SQ5,
                                         accum_out=slot(q, 2))

            def class_chunk(nc, q, j, sub_eng, mul_eng, red="A"):
                g = GRPS[q]
                w = CLS_CHUNKS[q][j]
                t4 = t4t[q][:, :, :]
                e = tcl[q][j]
                sub_eng.tensor_tensor(e, pc[q][j], e, OP.subtract)
                t4b = t4.unsqueeze(2).broadcast_to([P, g, w, SS])
                if red == "A":
                    mul_eng.tensor_tensor(e, e, t4b, OP.mult)
                    nc.scalar.activation(e, e, AF.Square, scale=1.0,
                                         accum_out=slot(q, 3 + j))
                else:
                    # reduce on DVE: me goes to the pc tile (pred dead
                    # after sub), then se = me*e reduced via tensor_reduce
                    me = pc[q][j]
                    mul_eng.tensor_tensor(me, e, t4b, OP.mult)
                    nc.vector.tensor_tensor(e, me, e, OP.mult)
                    nc.vector.tensor_reduce(
                        slot(q, 3 + j),
                        e[:, :, :, :].rearrange("p g c s -> p (g c s)"),
                        mybir.AxisListType.XY, OP.add)

            V, G = nc.vector, nc.gpsimd

            # ---- emission order (per-engine queues are in-order),
            # ---- interleaved by expected readiness ----
            def cls(q, j, tab):
                ent = tab[j]
                se, me = ent[0], ent[1]
                red = ent[2] if len(ent) > 2 else "A"
                class_chunk(nc, q, j, G if se == "G" else V,
                            G if me == "G" else V, red=red)

            def coord(q):
                se, me = COORD_ENG[q]
                coord_phase(nc, q, G if se == "G" else V,
                            G if me == "G" else V)

            box_phase(nc, 0)
            box_phase(nc, 1)
            coord(0)
            class_chunk(nc, 0, 0, G, V)
            class_chunk(nc, 0, 1, G, V)
            coord(1)
            class_chunk(nc, 0, 2, G, V)
            class_chunk(nc, 0, 3, G, V)
            class_chunk(nc, 0, 4, G, V)
            for j in range(len(CLS_CHUNKS[1])):
                cls(1, j, Q1_ENG)
                if j == BOX2_EMIT:
                    box_phase(nc, 2)
                    coord(2)
            for j in range(len(CLS_CHUNKS[2])):
                cls(2, j, Q2_ENG)

            nc.sync.dma_start(out=out[:, :], in_=acc)
    nc.compile()
    return nc


def _get_nc():
    if "nc" not in _CACHE:
        _CACHE["nc"] = _build()
    return _CACHE["nc"]


def kernel(pred: np.ndarray, target: np.ndarray) -> np.ndarray:
    nc = _get_nc()
    in_maps = []
    for k in range(NCORE):
        sl = slice(k * NPC, (k + 1) * NPC)
        in_maps.append({
            "pred": np.ascontiguousarray(pred[sl]).reshape(NPC, D * SS),
            "target": np.ascontiguousarray(target[sl]).reshape(NPC, D * SS),
        })
    res = run_bass_kernel_spmd(nc, in_maps, core_ids=list(range(NCORE)))
    total = sum(float(r["out"].astype(np.float64).sum()) for r in res.results)
    return np.float32(total / N)


# revision 41
# speedup vs baseline: 1.0052x; 1.0052x over previous
"""YOLO loss kernel for Trainium2 (Bass/Tile), data-parallel over 8 NeuronCores.

Math (per sample n, cell s; S=14, SS=196, B=2, C=20, D=30):
  t4 = target conf channel (binary 0/1, channels 4 and 9 identical)
  All box coords scaled by S (iou is invariant): corner = x +- (S/2)w,
  areas = S^2 * w * h.
  For the box PAIR (channels {o..o+3} for o in {0,5}, strided APs):
    prb = x + (S/2)w ; nlt = (S/2)w - x   (= -lt)
    ox = min(tprb, prb) + min(tnlt, nlt) ; oy likewise
    inter = relu(ox)*max(oy,0) ; union = S^2*pw*ph + S^2*tw*th - inter
    iou = inter * recip(union)
  sel = iou1 > iou0 ; selm = sel*t4 ; s0m = t4 - selm  (mask pair msk=[s0m,selm])
  conf  = sum msk_h*(p_{4+5h} - iou_h)^2          (pair op, channels {4,9})
  noobj = sum 0.5*(1-t4)*(p4^2 + p9^2)            (pair op)
  coord = 5 * sum_k msk_h*(p_{5h+k}-t_{5h+k})^2   (8-channel pair op, bcast msk)
  class = sum t4 * (p_c-t_c)^2, c=10..29          (4-channel chunks, bcast t4)
Masked squares use (mask*e)^2 = mask*e^2 (masks binary); weights fold into the
ACT Square scale. Every reduction is an ACT accum_out into a [128, NSLOT]
partial tile; host sums across slots/partitions/cores and divides by N.

Layout per core: 512 samples = 4 blocks x 128 partitions, processed in passes
of GRPS=[1,2,1] blocks (small first pass -> compute starts early; small last
pass + small trailing class chunks -> short drain tail). Engine split per the
TimelineSim cost model: DVE all stt/min/cmp + chain ops, Pool (gpsimd, 0.42
efficiency on add/sub/mult, nothing else supported) the independent big
sub/mults, ACT the square-accumulate reductions; the final chunk reduces via
mul+tensor_reduce on DVE to dodge the tail ACT queue. DMA stream is ordered so
it never stalls (65.8us = bytes/360GBps, gap-free): box data for passes 0/1,
class 0, one class-1 chunk, box data for pass 2, rest of class 1, class 2;
per-tag SBUF ring depths sized so no buffer-free wait ever blocks the queue.
"""

import numpy as np

import concourse.mybir as mybir
from concourse import bacc
from concourse.bass_utils import run_bass_kernel_spmd
from concourse.tile import TileContext

F32 = mybir.dt.float32
OP = mybir.AluOpType
AF = mybir.ActivationFunctionType

N, D, S = 4096, 30, 14
SS = S * S          # 196
NCORE = 8
NPC = N // NCORE    # 512 samples per core
P = 128
NBLK = NPC // P     # 4 blocks of 128 samples
GRPS = [1, 2, 1]    # blocks per pass
HALF = S / 2.0      # corner scale
AREA = float(S * S)
SQ5 = 5.0 ** 0.5
SQH = 0.5 ** 0.5

# class chunks (channel counts); last pass splits the tail chunk
CLS_CHUNKS = [[4, 4, 4, 4, 4], [4, 4, 4, 4, 4], [4, 4, 4, 4, 2, 2]]
BOX2_AFTER = 0
SMALL_BUFS = 3
PC1_BUFS = 5
TC1_BUFS = 5
TC2_BUFS = 4
COORD_SPLIT = ()
CLS0_EARLY = 0
BOX2_EMIT = 0
COORD_ENG = [("G", "V"), ("G", "V"), ("G", "V")]
NM_ENG = ["G", "G", "G"]
PQ_ENG = ["G", "G", "G"]
Q1_ENG = [("G", "V"), ("G", "V"), ("G", "G"), ("G", "V"), ("G", "V")]
Q2_ENG = [("G", "V"), ("G", "V"), ("G", "V"), ("G", "V"), ("G", "G"),
          ("V", "V", "B")]
SLOTS_PER_PASS = [3 + len(c) for c in CLS_CHUNKS]   # noobj, conf, coord + class
NSLOT = sum(SLOTS_PER_PASS) + 5   # last slot widened to 6 bn_stats values

_CACHE = {}


def _build():
    nc = bacc.Bacc("TRN2", target_bir_lowering=False, debug=False)
    pred = nc.dram_tensor("pred", [NPC, D * SS], F32, kind="ExternalInput")
    tgt = nc.dram_tensor("target", [NPC, D * SS], F32, kind="ExternalInput")
    out = nc.dram_tensor("out", [P, NSLOT], F32, kind="ExternalOutput")

    # [NPC, D*SS] -> [P, block, D*SS]; sample = block*128 + p
    pred_r = pred[:, :].rearrange("(a p) d -> p a d", a=NBLK)
    tgt_r = tgt[:, :].rearrange("(a p) d -> p a d", a=NBLK)

    npass = len(GRPS)
    offs = [sum(GRPS[:q]) for q in range(npass)]

    with TileContext(nc) as tc:
        with (
            tc.tile_pool(name="box", bufs=1) as boxp,     # pb/tb per pass
            tc.tile_pool(name="cls", bufs=3) as clsp,     # class chunk streams
            tc.tile_pool(name="tmp", bufs=1) as tmp,      # per-pass temps
            tc.tile_pool(name="accp", bufs=1) as accp,
        ):
            acc = accp.tile([P, NSLOT], F32)

            pb, tb, t4t = [], [], []
            pc, tcl = [], []
            for q, g in enumerate(GRPS):
                pb.append(boxp.tile([P, g, 10, SS], F32, tag=f"pb{q}",
                                    name=f"pb{q}"))
                # target as 8-ch box tile {0..3,5..8} + separate t4 tile
                tb.append(boxp.tile([P, g, 2, 4, SS], F32, tag=f"tb{q}",
                                    name=f"tb{q}"))
                t4t.append(boxp.tile([P, g, SS], F32, tag=f"t4_{q}",
                                     name=f"t4_{q}"))
                pcq, tcq = [], []
                for j, w in enumerate(CLS_CHUNKS[q]):
                    pbufs = (PC1_BUFS if g == 1 else 3) if w == 4 else SMALL_BUFS
                    tbufs = (TC1_BUFS if g == 1 else TC2_BUFS) if w == 4 else SMALL_BUFS
                    pcq.append(clsp.tile([P, g, w, SS], F32, tag=f"pc{g}_{w}",
                                         name=f"pc{q}_{j}", bufs=pbufs))
                    tcq.append(clsp.tile([P, g, w, SS], F32, tag=f"tc{g}_{w}",
                                         name=f"tc{q}_{j}", bufs=tbufs))
                pc.append(pcq)
                tcl.append(tcq)

            def dma_box(q):
                g = GRPS[q]
                a0 = offs[q]
                tgt8 = tgt_r[:, a0:a0 + g, 0:10 * SS].rearrange(
                    "p g (h c s) -> p g h c s", h=2, c=5)
                for h in range(2):
                    nc.sync.dma_start(
                        out=tb[q][:, :, h, :, :], in_=tgt8[:, :, h, 0:4, :])
                nc.sync.dma_start(out=t4t[q],
                                  in_=tgt_r[:, a0:a0 + g, 4 * SS:5 * SS])
                nc.sync.dma_start(
                    out=pb[q], in_=pred_r[:, a0:a0 + g, 0:10 * SS])

            def dma_class(q, j, which="both"):
                g = GRPS[q]
                a0 = offs[q]
                lo = 10 + sum(CLS_CHUNKS[q][:j])
                w = CLS_CHUNKS[q][j]
                if which in ("both", "p"):
                    nc.sync.dma_start(
                        out=pc[q][j],
                        in_=pred_r[:, a0:a0 + g, lo * SS:(lo + w) * SS])
                if which in ("both", "t"):
                    nc.sync.dma_start(
                        out=tcl[q][j],
                        in_=tgt_r[:, a0:a0 + g, lo * SS:(lo + w) * SS])

            # DMA stream order: boxes q0/q1, class0, class1 (box q2
            # placed after class1 chunk BOX2_AFTER; -1 = upfront)
            dma_box(0)
            for j in range(CLS0_EARLY):
                dma_class(0, j)
            dma_box(1)
            if BOX2_AFTER < 0:
                dma_box(2)
            for j in range(CLS0_EARLY, len(CLS_CHUNKS[0])):
                dma_class(0, j)
            for j in range(len(CLS_CHUNKS[1])):
                dma_class(1, j)
                if j == BOX2_AFTER:
                    dma_box(2)
            for j in range(len(CLS_CHUNKS[2])):
                dma_class(2, j)

            # ---- per-pass state ----
            st = [dict() for _ in range(npass)]

            def slot(q, i):
                base = sum(SLOTS_PER_PASS[:q])
                return acc[:, base + i:base + i + 1]

            def T(q, name, shape):
                t = tmp.tile(shape, F32, tag=f"{name}g{GRPS[q]}",
                             name=f"{name}{q}")
                st[q][name] = t
                return t

            def box_phase(nc, q):
                g = GRPS[q]
                s = st[q]
                pbv = pb[q][:, :, :, :]
                tb0 = tb[q][:, :, 0, :, :]          # target box channels 0..3
                t4 = t4t[q][:, :, :]

                # target prep (DVE stt): scaled corners + area
                tpr = T(q, "tpr", [P, g, 2, SS])    # [x-axis, y-axis]
                tnl = T(q, "tnl", [P, g, 2, SS])
                nc.vector.scalar_tensor_tensor(
                    tpr, tb0[:, :, 2:4, :], HALF, tb0[:, :, 0:2, :],
                    OP.mult, OP.add)
                nc.vector.scalar_tensor_tensor(
                    tnl, tb0[:, :, 2:4, :], HALF, tb0[:, :, 0:2, :],
                    OP.mult, OP.subtract)
                tarea = T(q, "tarea", [P, g, SS])
                nc.vector.scalar_tensor_tensor(
                    tarea, tb0[:, :, 2, :], AREA, tb0[:, :, 3, :],
                    OP.mult, OP.mult)

                # w = 1 - t4 (ACT); noobj pair masked by w
                w = T(q, "w", [P, g, SS])
                nc.scalar.activation(w, t4, AF.Copy, bias=1.0, scale=-1.0)
                conf_p = pbv.rearrange("p g (h c) s -> p g h c s", h=2)[:, :, :, 4, :]
                nm = T(q, "ce", [P, g, 2, SS])
                wb2 = w[:, :, :].unsqueeze(2).broadcast_to([P, g, 2, SS])
                nm_eng = nc.gpsimd if NM_ENG[q] == "G" else nc.vector
                nm_eng.tensor_tensor(nm, conf_p, wb2, OP.mult)
                nc.scalar.activation(nm, nm, AF.Square, scale=SQH,
                                     accum_out=slot(q, 0))

                # pred corners, both boxes at once: channel pairs {c, c+5}
                pv = pbv.rearrange("p g (h c) s -> p g h c s", h=2)
                pw_x = pv[:, :, :, 2, :]   # [P, g, 2(box), SS]
                pw_y = pv[:, :, :, 3, :]
                px = pv[:, :, :, 0, :]
                py = pv[:, :, :, 1, :]
                prx = T(q, "prx", [P, g, 2, SS])
                nlx = T(q, "nlx", [P, g, 2, SS])
                pry = T(q, "pry", [P, g, 2, SS])
                nly = T(q, "nly", [P, g, 2, SS])
                nc.vector.scalar_tensor_tensor(prx, pw_x, HALF, px, OP.mult, OP.add)
                nc.vector.scalar_tensor_tensor(nlx, pw_x, HALF, px, OP.mult, OP.subtract)
                nc.vector.scalar_tensor_tensor(pry, pw_y, HALF, py, OP.mult, OP.add)
                nc.vector.scalar_tensor_tensor(nly, pw_y, HALF, py, OP.mult, OP.subtract)
                # pq = pred area (Pool): (S^2 folded in s1 stt via AREA scale)
                pq = T(q, "pq", [P, g, 2, SS])
                pq_eng = nc.gpsimd if PQ_ENG[q] == "G" else nc.vector
                pq_eng.tensor_tensor(pq, pw_x, pw_y, OP.mult)

                # intersect: mins (target side broadcast over box pair)
                tprxb = tpr[:, :, 0, :].unsqueeze(2).broadcast_to([P, g, 2, SS])
                tpryb = tpr[:, :, 1, :].unsqueeze(2).broadcast_to([P, g, 2, SS])
                tnlxb = tnl[:, :, 0, :].unsqueeze(2).broadcast_to([P, g, 2, SS])
                tnlyb = tnl[:, :, 1, :].unsqueeze(2).broadcast_to([P, g, 2, SS])
                nc.vector.tensor_tensor(prx, tprxb, prx, OP.min)
                nc.vector.tensor_tensor(nlx, tnlxb, nlx, OP.min)
                nc.vector.tensor_tensor(pry, tpryb, pry, OP.min)
                nc.vector.tensor_tensor(nly, tnlyb, nly, OP.min)
                nc.vector.tensor_add(prx, prx, nlx)          # ox
                nc.vector.tensor_add(pry, pry, nly)          # oy
                nc.scalar.activation(nlx, prx, AF.Relu)      # relu(ox)
                nc.vector.scalar_tensor_tensor(
                    pry, pry, 0.0, nlx, OP.max, OP.mult)     # inter
                # union = (S^2*pq - inter) + tarea
                tareab = tarea[:, :, :].unsqueeze(2).broadcast_to([P, g, 2, SS])
                nc.vector.scalar_tensor_tensor(
                    nly, pq, AREA, pry, OP.mult, OP.subtract)
                nc.vector.tensor_tensor(nly, nly, tareab, OP.add)    # union
                nc.vector.reciprocal_approx_fast(
                    out=prx[:, :, :, :].rearrange("p g h s -> p (g h s)"),
                    in_=nly[:, :, :, :].rearrange("p g h s -> p (g h s)"))
                iou = pq
                st[q]["iou"] = iou
                nc.vector.tensor_mul(iou, pry, prx)

                # masks
                sel = tarea
                nc.vector.tensor_tensor(
                    sel, iou[:, :, 1, :], iou[:, :, 0, :], OP.is_gt)
                msk = s["tpr"]
                st[q]["msk"] = msk
                nc.vector.tensor_mul(msk[:, :, 1, :], sel, t4)       # selm
                nc.vector.tensor_sub(msk[:, :, 0, :], t4, msk[:, :, 1, :])  # s0m

                # conf pair: (p_conf - iou) * msk
                ce = s["ce"]
                nc.vector.tensor_sub(ce, conf_p, iou)
                nc.vector.tensor_mul(ce, ce, msk)
                nc.scalar.activation(ce, ce, AF.Square, scale=1.0,
                                     accum_out=slot(q, 1))

            def coord_phase(nc, q, sub_eng, mul_eng):
                g = GRPS[q]
                s = st[q]
                pv8 = pb[q][:, :, :, :].rearrange(
                    "p g (h c) s -> p g h c s", h=2)[:, :, :, 0:4, :]
                tv8 = tb[q][:, :, :, :, :]
                e8 = T(q, "e8", [P, g, 2, 4, SS])
                mskb = s["msk"][:, :, :, :].unsqueeze(3).broadcast_to(
                    [P, g, 2, 4, SS])
                if q in COORD_SPLIT:
                    # independent half-chains (h=0: s0m/box0, h=1: selm/box1)
                    for h in range(2):
                        sub_eng.tensor_tensor(
                            e8[:, :, h, :, :], pv8[:, :, h, :, :],
                            tv8[:, :, h, :, :], OP.subtract)
                        mul_eng.tensor_tensor(
                            e8[:, :, h, :, :], e8[:, :, h, :, :],
                            mskb[:, :, h, :, :], OP.mult)
                        nc.scalar.activation(
                            e8[:, :, h, :, :], e8[:, :, h, :, :], AF.Square,
                            scale=SQ5, accum_out=slot(q, 2) if h == 0
                            else acc[:, NSLOT - 1 - q:NSLOT - q])
                else:
                    sub_eng.tensor_tensor(e8, pv8, tv8, OP.subtract)
                    mul_eng.tensor_tensor(e8, e8, mskb, OP.mult)
                    nc.scalar.activation(e8, e8, AF.Square, scale=SQ5,
                                         accum_out=slot(q, 2))

            def class_chunk(nc, q, j, sub_eng, mul_eng, red="A"):
                g = GRPS[q]
                w = CLS_CHUNKS[q][j]
                t4 = t4t[q][:, :, :]
                e = tcl[q][j]
                sub_eng.tensor_tensor(e, pc[q][j], e, OP.subtract)
                t4b = t4.unsqueeze(2).broadcast_to([P, g, w, SS])
                if red == "A":
                    mul_eng.tensor_tensor(e, e, t4b, OP.mult)
                    nc.scalar.activation(e, e, AF.Square, scale=1.0,
                                         accum_out=slot(q, 3 + j))
                elif red == "B":
                    # single-op reduce: bn_stats(me) -> (n, mean, M2) x2;
                    # host recovers sum(me^2) = sum_g(n*mean^2 + M2)
                    mul_eng.tensor_tensor(e, e, t4b, OP.mult)
                    base = sum(SLOTS_PER_PASS[:q]) + 3 + j
                    nc.vector.bn_stats(
                        acc[:, base:base + 6],
                        e[:, :, :, :].rearrange("p g c s -> p (g c s)"))
                else:
                    # reduce on DVE: me goes to the pc tile (pred dead
                    # after sub), then se = me*e reduced via tensor_reduce
                    me = pc[q][j]
                    mul_eng.tensor_tensor(me, e, t4b, OP.mult)
                    nc.vector.tensor_tensor(e, me, e, OP.mult)
                    nc.vector.tensor_reduce(
                        slot(q, 3 + j),
                        e[:, :, :, :].rearrange("p g c s -> p (g c s)"),
                        mybir.AxisListType.XY, OP.add)

            V, G = nc.vector, nc.gpsimd

            # ---- emission order (per-engine queues are in-order),
            # ---- interleaved by expected readiness ----
            def cls(q, j, tab):
                ent = tab[j]
                se, me = ent[0], ent[1]
                red = ent[2] if len(ent) > 2 else "A"
                class_chunk(nc, q, j, G if se == "G" else V,
                            G if me == "G" else V, red=red)

            def coord(q):
                se, me = COORD_ENG[q]
                coord_phase(nc, q, G if se == "G" else V,
                            G if me == "G" else V)

            box_phase(nc, 0)
            box_phase(nc, 1)
            coord(0)
            class_chunk(nc, 0, 0, G, V)
            class_chunk(nc, 0, 1, G, V)
            coord(1)
            class_chunk(nc, 0, 2, G, V)
            class_chunk(nc, 0, 3, G, V)
            class_chunk(nc, 0, 4, G, V)
            for j in range(len(CLS_CHUNKS[1])):
                cls(1, j, Q1_ENG)
                if j == BOX2_EMIT:
                    box_phase(nc, 2)
                    coord(2)
            for j in range(len(CLS_CHUNKS[2])):
                cls(2, j, Q2_ENG)

            nc.sync.dma_start(out=out[:, :], in_=acc)
    nc.compile()
    return nc


def _get_nc():
    if "nc" not in _CACHE:
        _CACHE["nc"] = _build()
    return _CACHE["nc"]


def kernel(pred: np.ndarray, target: np.ndarray) -> np.ndarray:
    nc = _get_nc()
    in_maps = []
    for k in range(NCORE):
        sl = slice(k * NPC, (k + 1) * NPC)
        in_maps.append({
            "pred": np.ascontiguousarray(pred[sl]).reshape(NPC, D * SS),
            "target": np.ascontiguousarray(target[sl]).reshape(NPC, D * SS),
        })
    res = run_bass_kernel_spmd(nc, in_maps, core_ids=list(range(NCORE)))
    total = 0.0
    for r in res.results:
        o = r["out"].astype(np.float64)
        total += float(o[:, :NSLOT - 6].sum())
        s = o[:, NSLOT - 6:]
        total += float((s[:, 0] * s[:, 1] ** 2 + s[:, 2]
                        + s[:, 3] * s[:, 4] ** 2 + s[:, 5]).sum())
    return np.float32(total / N)
